# revision 2
# baseline (speedup 1.0000x reference)
"""Trainium2 Bass kernel for nn_Lut3D: 3D LUT trilinear interpolation.

Data-parallel over 8 NeuronCores (2 batches/core). The interpolation runs
ON DEVICE:
  P1: from uint16-quantized input planes compute, per pixel, the compact
      cell index base=(kb*32+kg)*32+kr (int16) and four fp16 weight-pairs
      u_j = (w_j*(1-fr), w_j*fr) for the 4 (blue,green)-corner combos.
  P2: ap_gather fetches per pixel the fp16 (r, r+1) value pair for all 12
      (channel, corner) tables at once (tables live per 16-partition group,
      rows t=c*4+j); P = G * U elementwise; PE matmul with a 0/1 selector
      sums the 8 corner terms per channel; fp16 result planes DMA out.

Host only quantizes x to uint16 (halves the upload), builds the tiny packed
tables, and converts the fp16 output back to fp32.
"""

import os
import sys

import numpy as np

os.environ.setdefault("NEURON_RT_RESET_CORES", "1")
sys.path.insert(0, "/opt/trn_rl_repo")

import concourse.bass as bass  # noqa: E402
import concourse.tile as tile  # noqa: E402
from concourse import bacc, mybir  # noqa: E402
from concourse.bass_utils import run_bass_kernel_spmd  # noqa: E402

B, C, H, W = 16, 3, 1080, 1920
N_CORES = 8
B_SH = B // N_CORES                      # 2 batches per core
PLANE = H * W                            # 2,073,600 px per (batch, channel)
PX = B_SH * PLANE                        # 4,147,200 px per core

# P1 tiling: natural [128, F1] tiles
F1 = 648
T1 = PLANE // (128 * F1)                 # 25 tiles per batch
N1 = 128 * F1                            # 82,944 px per P1 tile

# P2 tiling: per-group stream windows
SBAR = 324                               # idx cols per group
GW = 16 * SBAR                           # 5,184 px per group window
N2 = 8 * GW                              # 41,472 px per P2 tile
T2 = PLANE // N2                         # 50 tiles per batch
NCHUNK = 12                              # psum col chunks per tile
CH = GW // NCHUNK                        # 432 cols per chunk

CSCALE = np.float32(32.0 / 1.000001 / 65535.0)

_CACHED = {}


def _build_program():
    if "nc" in _CACHED:
        return _CACHED["nc"]
    nc = bacc.Bacc("TRN2", target_bir_lowering=False, debug=False,
                   num_devices=N_CORES)
    f32, f16, i16, i32, u16, u32 = (mybir.dt.float32, mybir.dt.float16,
                                    mybir.dt.int16, mybir.dt.int32,
                                    mybir.dt.uint16, mybir.dt.uint32)
    xq_d = nc.dram_tensor("xq", [6, PLANE], u16, kind="ExternalInput").ap()
    tab_d = nc.dram_tensor("tab", [16, 32768], u32, kind="ExternalInput").ap()
    sel_d = nc.dram_tensor("sel", [128, 24], f16, kind="ExternalInput").ap()
    out_d = nc.dram_tensor("out", [6, PLANE], f16, kind="ExternalOutput").ap()
    idx_d = nc.dram_tensor("idxbuf", [1, PX], i16, kind="Internal").ap()
    u2_d = nc.dram_tensor("u2buf", [4, 2 * PX], f16, kind="Internal").ap()

    with tile.TileContext(nc) as tc:
        with tc.tile_pool(name="tabs", bufs=1) as tpool:
            TAB = tpool.tile([128, 32768], u32)
            SEL = tpool.tile([128, 24], f16)
            for g in range(8):
                nc.sync.dma_start(TAB[16 * g:16 * (g + 1), :], tab_d)
            nc.sync.dma_start(SEL[:], sel_d)

            # ---------------- P1 ----------------
            with tc.tile_pool(name="p1", bufs=2) as pool:
                for b in range(B_SH):
                    with tc.For_i(0, T1) as i:
                        xq = [pool.tile([128, F1], u16, tag=f"xq{c}")
                              for c in range(3)]
                        for c in range(3):
                            src = xq_d[3 * b + c, bass.ts(i, N1)]
                            nc.sync.dma_start(
                                xq[c][:],
                                src.rearrange("(p f) -> p f", p=128))
                        t_ = [pool.tile([128, F1], f32, tag=f"t{c}")
                              for c in range(3)]
                        kf = [pool.tile([128, F1], f32, tag=f"kf{c}")
                              for c in range(3)]
                        fr_ = [pool.tile([128, F1], f32, tag=f"fr{c}")
                               for c in range(3)]
                        ki = pool.tile([128, F1], i32, tag="ki")
                        corr = pool.tile([128, F1], f32, tag="corr")
                        for c in range(3):
                            nc.scalar.activation(
                                t_[c][:], xq[c][:],
                                mybir.ActivationFunctionType.Copy,
                                0.0, float(CSCALE))
                            nc.scalar.copy(ki[:], t_[c][:])
                            nc.scalar.copy(kf[c][:], ki[:])
                            nc.vector.tensor_tensor(
                                corr[:], kf[c][:], t_[c][:],
                                mybir.AluOpType.is_gt)
                            nc.vector.tensor_tensor(
                                kf[c][:], kf[c][:], corr[:],
                                mybir.AluOpType.subtract)
                            nc.vector.tensor_tensor(
                                fr_[c][:], t_[c][:], kf[c][:],
                                mybir.AluOpType.subtract)
                        # base = (kb*32+kg)*32+kr  (planes: r=0, g=1, b=2)
                        base = pool.tile([128, F1], f32, tag="base")
                        nc.vector.scalar_tensor_tensor(
                            base[:], kf[2][:], 32.0, kf[1][:],
                            mybir.AluOpType.mult, mybir.AluOpType.add)
                        nc.vector.scalar_tensor_tensor(
                            base[:], base[:], 32.0, kf[0][:],
                            mybir.AluOpType.mult, mybir.AluOpType.add)
                        bi16 = pool.tile([128, F1], i16, tag="bi16")
                        nc.scalar.copy(bi16[:], base[:])
                        nc.sync.dma_start(
                            idx_d[0, bass.ts(i + T1 * b, N1)].rearrange(
                                "(p f) -> p f", p=128),
                            bi16[:])
                        # weights: j=0:(1-fb)(1-fg) 1:(1-fb)fg 2:fb(1-fg) 3:fb*fg
                        fr, fg, fb = fr_[0], fr_[1], fr_[2]
                        w3 = pool.tile([128, F1], f32, tag="w3")
                        w1 = pool.tile([128, F1], f32, tag="w1")
                        w2 = pool.tile([128, F1], f32, tag="w2")
                        w0 = pool.tile([128, F1], f32, tag="w0")
                        nc.vector.tensor_tensor(
                            w3[:], fb[:], fg[:], mybir.AluOpType.mult)
                        nc.vector.tensor_tensor(
                            w1[:], fg[:], w3[:], mybir.AluOpType.subtract)
                        nc.vector.tensor_tensor(
                            w2[:], fb[:], w3[:], mybir.AluOpType.subtract)
                        nc.vector.tensor_tensor(
                            w0[:], fg[:], w2[:], mybir.AluOpType.add)
                        nc.vector.tensor_scalar(
                            w0[:], w0[:], -1.0, 1.0,
                            mybir.AluOpType.mult, mybir.AluOpType.add)
                        for j, wj in enumerate((w0, w1, w2, w3)):
                            u2 = pool.tile([128, 2 * F1], f16, tag=f"u2{j}")
                            uv = u2[:].rearrange("p (f two) -> p f two", two=2)
                            # odd = wj*fr ; even = wj - odd
                            nc.vector.tensor_tensor(
                                uv[:, :, 1], wj[:], fr[:],
                                mybir.AluOpType.mult)
                            nc.vector.tensor_tensor(
                                uv[:, :, 0], wj[:], uv[:, :, 1],
                                mybir.AluOpType.subtract)
                            nc.sync.dma_start(
                                u2_d[j, bass.ts(i + T1 * b, 2 * N1)].rearrange(
                                    "(p f) -> p f", p=128),
                                u2[:])

            # ---------------- P2 ----------------
            with tc.tile_pool(name="p2", bufs=1) as pool, \
                 tc.tile_pool(name="ps", bufs=4, space="PSUM") as pspool:
                for b in range(B_SH):
                    with tc.For_i(0, T2) as ii:
                        tt = ii + T2 * b
                        idxw = pool.tile([128, SBAR], i16, tag="idxw")
                        for g in range(8):
                            src = idx_d[0, bass.ds(tt * N2 + g * GW, GW)]
                            nc.sync.dma_start(
                                idxw[16 * g:16 * (g + 1), :],
                                src.rearrange("(s p) -> p s", p=16))
                        U = pool.tile([128, 2 * GW], f16, tag="U")
                        Uv = U[:].rearrange("(g s) f -> g s f", g=8)
                        for j in range(4):
                            src = u2_d[j, bass.ds(tt * 2 * N2, 2 * N2)]
                            srcv = src.rearrange("(g f) -> g f", g=8)
                            for a in range(4):
                                nc.sync.dma_start(Uv[:, 4 * a + j, :], srcv)
                        G = pool.tile([128, GW], u32, tag="G")
                        nc.gpsimd.ap_gather(
                            G[:], TAB[:], idxw[:], channels=128,
                            num_elems=32768, d=1, num_idxs=GW)
                        P = pool.tile([128, 2 * GW], f16, tag="P")
                        nc.vector.tensor_tensor(
                            P[:], G[:].bitcast(mybir.dt.float16), U[:],
                            mybir.AluOpType.mult)
                        Pv = P[:].rearrange("p (n two) -> p n two", two=2)
                        for h in range(NCHUNK):
                            ps = pspool.tile([24, CH], mybir.dt.float32,
                                             tag=f"ps{h % 4}")
                            nc.tensor.matmul(
                                ps[:], SEL[:],
                                Pv[:, h * CH:(h + 1) * CH, 0],
                                start=True, stop=False)
                            nc.tensor.matmul(
                                ps[:], SEL[:],
                                Pv[:, h * CH:(h + 1) * CH, 1],
                                start=False, stop=True)
                            osb = pool.tile([24, CH], f16, tag=f"osb{h % 4}")
                            nc.scalar.copy(osb[:], ps[:])
                            dst = out_d[3 * b:3 * b + 3,
                                        bass.ts(ii, N2)].rearrange(
                                "c (g s) -> c g s", g=8)[
                                :, :, h * CH:(h + 1) * CH]
                            nc.sync.dma_start(
                                dst,
                                osb[:].rearrange("(c g) s -> c g s", c=3))
    nc.compile()
    _CACHED["nc"] = nc
    return nc


def _build_tables(lut):
    """tab[16, 32768] uint32: row t=c*4+(db*2+dg): pack(L16[b+db,g+dg,r],
    L16[b+db,g+dg,r+1]); rows 12-15 replicate rows 8-11."""
    l16 = lut.astype(np.float16).view(np.uint16)  # [3, 33, 33, 33]
    tab = np.zeros((16, 32768), dtype=np.uint32)
    for c in range(3):
        for db in range(2):
            for dg in range(2):
                t = c * 4 + db * 2 + dg
                lo = l16[c, db:db + 32, dg:dg + 32, 0:32].astype(np.uint32)
                hi = l16[c, db:db + 32, dg:dg + 32, 1:33].astype(np.uint32)
                tab[t] = (lo | (hi << 16)).reshape(32768)
    tab[12:16] = tab[8:12]
    return tab


def _build_sel():
    sel = np.zeros((128, 24), dtype=np.float16)
    for g in range(8):
        for c in range(3):
            for j in range(4):
                sel[16 * g + c * 4 + j, c * 8 + g] = 1.0
    return sel


def kernel(lut, x):
    lut = np.ascontiguousarray(np.asarray(lut, dtype=np.float32))
    x = np.asarray(x, dtype=np.float32)

    tmp = x * np.float32(65535.0)
    np.add(tmp, np.float32(0.5), out=tmp)
    xq = tmp.astype(np.uint16)

    tab = _build_tables(lut)
    sel = _build_sel()

    nc = _build_program()
    in_maps = []
    for k in range(N_CORES):
        shard = xq[k * B_SH:(k + 1) * B_SH].reshape(6, PLANE)
        in_maps.append({"xq": np.ascontiguousarray(shard),
                        "tab": tab, "sel": sel})
    try:
        res = run_bass_kernel_spmd(nc, in_maps, list(range(N_CORES)))
    except Exception:
        res = run_bass_kernel_spmd(nc, in_maps, list(range(N_CORES)))
    outs = [res.results[k]["out"].reshape(B_SH, C, H, W)
            for k in range(N_CORES)]
    return np.concatenate(outs, axis=0).astype(np.float32)


if __name__ == "__main__":
    rng = np.random.default_rng(0)
    lut = rng.random((3, 33, 33, 33), dtype=np.float32)
    x = rng.random((B, C, H, W), dtype=np.float32)
    out = kernel(lut, x)
    print("out", out.shape, out.dtype, float(out.mean()))


# revision 3
# speedup vs baseline: 3.9163x; 3.9163x over previous
"""Trainium2 Bass kernel for nn_Lut3D: 3D LUT trilinear interpolation.

Data-parallel over 8 NeuronCores (2 batches/core). The interpolation runs
ON DEVICE:
  P1: from uint16-quantized input planes compute, per pixel, the compact
      cell index base=(kb*32+kg)*32+kr (int16) and four fp16 weight-pairs
      u_j = (w_j*(1-fr), w_j*fr) for the 4 (blue,green)-corner combos.
  P2: ap_gather fetches per pixel the fp16 (r, r+1) value pair for all 12
      (channel, corner) tables at once (tables live per 16-partition group,
      rows t=c*4+j); P = G * U elementwise; PE matmul with a 0/1 selector
      sums the 8 corner terms per channel; fp16 result planes DMA out.

Host only quantizes x to uint16 (halves the upload), builds the tiny packed
tables, and converts the fp16 output back to fp32.
"""

import os
import sys

import numpy as np

os.environ.setdefault("NEURON_RT_RESET_CORES", "1")
sys.path.insert(0, "/opt/trn_rl_repo")

import concourse.bass as bass  # noqa: E402
import concourse.tile as tile  # noqa: E402
from concourse import bacc, mybir  # noqa: E402
from concourse.bass_utils import run_bass_kernel_spmd  # noqa: E402

B, C, H, W = 16, 3, 1080, 1920
N_CORES = 8
B_SH = B // N_CORES                      # 2 batches per core
PLANE = H * W                            # 2,073,600 px per (batch, channel)
PX = B_SH * PLANE                        # 4,147,200 px per core

# P1 tiling: natural [128, F1] tiles
F1 = 648
T1 = PLANE // (128 * F1)                 # 25 tiles per batch
N1 = 128 * F1                            # 82,944 px per P1 tile

# P2 tiling: per-group stream windows
SBAR = 324                               # idx cols per group
GW = 16 * SBAR                           # 5,184 px per group window
N2 = 8 * GW                              # 41,472 px per P2 tile
T2 = PLANE // N2                         # 50 tiles per batch
NCHUNK = 12                              # psum col chunks per tile
CH = GW // NCHUNK                        # 432 cols per chunk

CSCALE = np.float32(32.0 / 1.000001 / 65535.0)

_CACHED = {}


def _build_program():
    if "nc" in _CACHED:
        return _CACHED["nc"]
    nc = bacc.Bacc("TRN2", target_bir_lowering=False, debug=False,
                   num_devices=N_CORES)
    f32, f16, i16, i32, u16, u32 = (mybir.dt.float32, mybir.dt.float16,
                                    mybir.dt.int16, mybir.dt.int32,
                                    mybir.dt.uint16, mybir.dt.uint32)
    xq_d = nc.dram_tensor("xq", [6, PLANE], u16, kind="ExternalInput").ap()
    tab_d = nc.dram_tensor("tab", [16, 32768], u32, kind="ExternalInput").ap()
    sel_d = nc.dram_tensor("sel", [128, 24], f16, kind="ExternalInput").ap()
    out_d = nc.dram_tensor("out", [6, PLANE], f16, kind="ExternalOutput").ap()
    idx_d = nc.dram_tensor("idxbuf", [1, PX], i16, kind="Internal").ap()
    u2_d = nc.dram_tensor("u2buf", [4, 2 * PX], f16, kind="Internal").ap()

    with tile.TileContext(nc) as tc:
        with tc.tile_pool(name="tabs", bufs=1) as tpool:
            TAB = tpool.tile([128, 32768], u32)
            SEL = tpool.tile([128, 24], f16)
            for g in range(8):
                nc.sync.dma_start(TAB[16 * g:16 * (g + 1), :], tab_d)
            nc.sync.dma_start(SEL[:], sel_d)

            # ---------------- P1 ----------------
            with tc.tile_pool(name="p1", bufs=2) as pool:
                for b in range(B_SH):
                    with tc.For_i(0, T1) as i:
                        xq = [pool.tile([128, F1], u16, tag=f"xq{c}")
                              for c in range(3)]
                        for c in range(3):
                            src = xq_d[3 * b + c, bass.ts(i, N1)]
                            nc.sync.dma_start(
                                xq[c][:],
                                src.rearrange("(p f) -> p f", p=128))
                        t_ = [pool.tile([128, F1], f32, tag=f"t{c}")
                              for c in range(3)]
                        kf = [pool.tile([128, F1], f32, tag=f"kf{c}")
                              for c in range(3)]
                        fr_ = [pool.tile([128, F1], f32, tag=f"fr{c}")
                               for c in range(3)]
                        ki = pool.tile([128, F1], i32, tag="ki")
                        corr = pool.tile([128, F1], f32, tag="corr")
                        for c in range(3):
                            nc.scalar.activation(
                                t_[c][:], xq[c][:],
                                mybir.ActivationFunctionType.Copy,
                                0.0, float(CSCALE))
                            nc.scalar.copy(ki[:], t_[c][:])
                            nc.scalar.copy(kf[c][:], ki[:])
                            nc.vector.tensor_tensor(
                                corr[:], kf[c][:], t_[c][:],
                                mybir.AluOpType.is_gt)
                            nc.vector.tensor_tensor(
                                kf[c][:], kf[c][:], corr[:],
                                mybir.AluOpType.subtract)
                            nc.vector.tensor_tensor(
                                fr_[c][:], t_[c][:], kf[c][:],
                                mybir.AluOpType.subtract)
                        # base = (kb*32+kg)*32+kr  (planes: r=0, g=1, b=2)
                        base = pool.tile([128, F1], f32, tag="base")
                        nc.vector.scalar_tensor_tensor(
                            base[:], kf[2][:], 32.0, kf[1][:],
                            mybir.AluOpType.mult, mybir.AluOpType.add)
                        nc.vector.scalar_tensor_tensor(
                            base[:], base[:], 32.0, kf[0][:],
                            mybir.AluOpType.mult, mybir.AluOpType.add)
                        bi16 = pool.tile([128, F1], i16, tag="bi16")
                        nc.scalar.copy(bi16[:], base[:])
                        nc.sync.dma_start(
                            idx_d[0, bass.ts(i + T1 * b, N1)].rearrange(
                                "(p f) -> p f", p=128),
                            bi16[:])
                        # weights: j=0:(1-fb)(1-fg) 1:(1-fb)fg 2:fb(1-fg) 3:fb*fg
                        fr, fg, fb = fr_[0], fr_[1], fr_[2]
                        w3 = pool.tile([128, F1], f32, tag="w3")
                        w1 = pool.tile([128, F1], f32, tag="w1")
                        w2 = pool.tile([128, F1], f32, tag="w2")
                        w0 = pool.tile([128, F1], f32, tag="w0")
                        nc.vector.tensor_tensor(
                            w3[:], fb[:], fg[:], mybir.AluOpType.mult)
                        nc.vector.tensor_tensor(
                            w1[:], fg[:], w3[:], mybir.AluOpType.subtract)
                        nc.vector.tensor_tensor(
                            w2[:], fb[:], w3[:], mybir.AluOpType.subtract)
                        nc.vector.tensor_tensor(
                            w0[:], fg[:], w2[:], mybir.AluOpType.add)
                        nc.vector.tensor_scalar(
                            w0[:], w0[:], -1.0, 1.0,
                            mybir.AluOpType.mult, mybir.AluOpType.add)
                        for j, wj in enumerate((w0, w1, w2, w3)):
                            u2 = pool.tile([128, 2 * F1], f16, tag=f"u2{j}")
                            uv = u2[:].rearrange("p (f two) -> p f two", two=2)
                            # odd = wj*fr ; even = wj - odd
                            nc.vector.tensor_tensor(
                                uv[:, :, 1], wj[:], fr[:],
                                mybir.AluOpType.mult)
                            nc.vector.tensor_tensor(
                                uv[:, :, 0], wj[:], uv[:, :, 1],
                                mybir.AluOpType.subtract)
                            nc.sync.dma_start(
                                u2_d[j, bass.ts(i + T1 * b, 2 * N1)].rearrange(
                                    "(p f) -> p f", p=128),
                                u2[:])

            # ---------------- P2 ----------------
            with tc.tile_pool(name="p2", bufs=1) as pool, \
                 tc.tile_pool(name="ps", bufs=4, space="PSUM") as pspool:
                for b in range(B_SH):
                    with tc.For_i(0, T2) as ii:
                        tt = ii + T2 * b
                        idxw = pool.tile([128, SBAR], i16, tag="idxw")
                        for g in range(8):
                            src = idx_d[0, bass.ds(tt * N2 + g * GW, GW)]
                            nc.sync.dma_start(
                                idxw[16 * g:16 * (g + 1), :],
                                src.rearrange("(s p) -> p s", p=16))
                        U = pool.tile([128, 2 * GW], f16, tag="U")
                        Uv = U[:].rearrange("(g s) f -> g s f", g=8)
                        for j in range(4):
                            src = u2_d[j, bass.ds(tt * 2 * N2, 2 * N2)]
                            srcv = src.rearrange("(g f) -> g f", g=8)
                            for a in range(4):
                                nc.sync.dma_start(Uv[:, 4 * a + j, :], srcv)
                        G = pool.tile([128, GW], u32, tag="G")
                        nc.gpsimd.ap_gather(
                            G[:], TAB[:], idxw[:], channels=128,
                            num_elems=32768, d=1, num_idxs=GW)
                        P = pool.tile([128, 2 * GW], f16, tag="P")
                        nc.vector.tensor_tensor(
                            P[:], G[:].bitcast(mybir.dt.float16), U[:],
                            mybir.AluOpType.mult)
                        Pv = P[:].rearrange("p (n two) -> p n two", two=2)
                        for h in range(NCHUNK):
                            ps = pspool.tile([24, CH], mybir.dt.float32,
                                             tag=f"ps{h % 4}")
                            nc.tensor.matmul(
                                ps[:], SEL[:],
                                Pv[:, h * CH:(h + 1) * CH, 0],
                                start=True, stop=False)
                            nc.tensor.matmul(
                                ps[:], SEL[:],
                                Pv[:, h * CH:(h + 1) * CH, 1],
                                start=False, stop=True)
                            osb = pool.tile([24, CH], f16, tag=f"osb{h % 4}")
                            nc.scalar.copy(osb[:], ps[:])
                            dst = out_d[3 * b:3 * b + 3,
                                        bass.ts(ii, N2)].rearrange(
                                "c (g s) -> c g s", g=8)[
                                :, :, h * CH:(h + 1) * CH]
                            nc.sync.dma_start(
                                dst,
                                osb[:].rearrange("(c g) s -> c g s", c=3))
    nc.compile()
    _CACHED["nc"] = nc
    return nc


def _build_tables(lut):
    """tab[16, 32768] uint32: row t=c*4+(db*2+dg): pack(L16[b+db,g+dg,r],
    L16[b+db,g+dg,r+1]); rows 12-15 replicate rows 8-11."""
    l16 = lut.astype(np.float16).view(np.uint16)  # [3, 33, 33, 33]
    tab = np.zeros((16, 32768), dtype=np.uint32)
    for c in range(3):
        for db in range(2):
            for dg in range(2):
                t = c * 4 + db * 2 + dg
                lo = l16[c, db:db + 32, dg:dg + 32, 0:32].astype(np.uint32)
                hi = l16[c, db:db + 32, dg:dg + 32, 1:33].astype(np.uint32)
                tab[t] = (lo | (hi << 16)).reshape(32768)
    tab[12:16] = tab[8:12]
    return tab


def _build_sel():
    sel = np.zeros((128, 24), dtype=np.float16)
    for g in range(8):
        for c in range(3):
            for j in range(4):
                sel[16 * g + c * 4 + j, c * 8 + g] = 1.0
    return sel


def kernel(lut, x):
    import time as _time
    _t0 = _time.perf_counter()
    lut = np.ascontiguousarray(np.asarray(lut, dtype=np.float32))
    x = np.asarray(x, dtype=np.float32)

    tmp = x * np.float32(65535.0)
    np.add(tmp, np.float32(0.5), out=tmp)
    xq = tmp.astype(np.uint16)

    tab = _build_tables(lut)
    sel = _build_sel()

    nc = _build_program()
    _t1 = _time.perf_counter()
    print(f"[kernel] prep+compile: {_t1-_t0:.2f}s", flush=True)
    in_maps = []
    for k in range(N_CORES):
        shard = xq[k * B_SH:(k + 1) * B_SH].reshape(6, PLANE)
        in_maps.append({"xq": np.ascontiguousarray(shard),
                        "tab": tab, "sel": sel})
    try:
        res = run_bass_kernel_spmd(nc, in_maps, list(range(N_CORES)))
    except Exception as e:
        print(f"[kernel] first run failed ({type(e).__name__}); retrying",
              flush=True)
        res = run_bass_kernel_spmd(nc, in_maps, list(range(N_CORES)))
    _t2 = _time.perf_counter()
    print(f"[kernel] device run: {_t2-_t1:.2f}s", flush=True)
    outs = [res.results[k]["out"].reshape(B_SH, C, H, W)
            for k in range(N_CORES)]
    return np.concatenate(outs, axis=0).astype(np.float32)


if __name__ == "__main__":
    rng = np.random.default_rng(0)
    lut = rng.random((3, 33, 33, 33), dtype=np.float32)
    x = rng.random((B, C, H, W), dtype=np.float32)
    out = kernel(lut, x)
    print("out", out.shape, out.dtype, float(out.mean()))
